# revision 11
# baseline (speedup 1.0000x reference)
"""Trainium2 Bass kernel for nn_Block_32762010534337 (dense transformer block).

Strategy: sequence-parallel over 8 cores. Core c owns 512 tokens (batch c//4,
token chunk c%4). Each core computes rmsnorm -> K/V projections (+rope, k-norm)
for its own tokens, AllGathers K/V within its batch group of 4 cores (overlapped
with the Q projections), then runs causal attention + wo + MLP (relu^2) for its
512 tokens with fully replicated bf16 weights. Activations stay feature-major
([channel, token]); the host transposes per-core inputs and the final residual
path switches to token-major via 64 PE transposes.
"""
import sys
import os

if "/opt/trn_rl_repo" not in sys.path:
    sys.path.insert(0, "/opt/trn_rl_repo")

import numpy as np

B, T, C = 2, 2048, 2048
NH, NKV, HD = 16, 4, 128
DFF = 4 * C
TQ = 512          # tokens per core
NT = C // 128     # 16 feature tiles
NF = DFF // 128   # 64 ff tiles
EPS = 1.1920929e-07
NCORES = 8

_CACHE = None


def _build():
    import concourse.bass as bass
    import concourse.tile as tile
    from concourse import mybir, bacc
    from concourse.masks import make_identity

    dt = mybir.dt
    f32, bf16 = dt.float32, dt.bfloat16
    Alu = mybir.AluOpType
    Act = mybir.ActivationFunctionType

    nc = bacc.Bacc("TRN2", target_bir_lowering=False, debug=False, num_devices=NCORES)

    for val in (EPS, HD * EPS):
        tns = nc.alloc_sbuf_tensor(f"const-f32-{val}", [128, 1], f32)
        nc.gpsimd.memset(tns.ap(), val)
        nc.const_aps.aps[(f32, val)] = tns.ap()
    nc.all_engine_barrier()

    xT = nc.declare_dram_parameter("xT", [C, TQ], f32, isOutput=False)
    csc = nc.declare_dram_parameter("csc", [128, TQ], f32, isOutput=False)
    css = nc.declare_dram_parameter("css", [128, TQ], f32, isOutput=False)
    mask = nc.declare_dram_parameter("mask", [T, TQ], bf16, isOutput=False)
    wq = nc.declare_dram_parameter("wq", [C, C], bf16, isOutput=False)
    wk = nc.declare_dram_parameter("wk", [C, NKV * HD], bf16, isOutput=False)
    wv = nc.declare_dram_parameter("wv", [C, NKV * HD], bf16, isOutput=False)
    wo = nc.declare_dram_parameter("wo", [C, C], bf16, isOutput=False)
    wfc = nc.declare_dram_parameter("wfc", [C, DFF], bf16, isOutput=False)
    wproj = nc.declare_dram_parameter("wproj", [DFF, C], bf16, isOutput=False)
    out_tm = nc.declare_dram_parameter("out", [TQ, C], f32, isOutput=True)

    ck_in = nc.dram_tensor("ck_in", [512, TQ], bf16)
    ck_out = nc.dram_tensor("ck_out", [2048, TQ], bf16)
    cv_in = nc.dram_tensor("cv_in", [512, TQ], bf16)
    cv_out = nc.dram_tensor("cv_out", [2048, TQ], bf16)

    with tile.TileContext(nc, num_cores=NCORES) as tc:
        with (
            tc.tile_pool(name="const", bufs=1) as constp,
            tc.tile_pool(name="persist", bufs=1) as pp,
            tc.tile_pool(name="work", bufs=3) as wpool,
            tc.tile_pool(name="wstream", bufs=3) as wsp,
        ):
            ident = constp.tile([128, 128], bf16, tag="ident")
            make_identity(nc, ident)
            ident_f = constp.tile([128, 128], f32, tag="identf")
            make_identity(nc, ident_f)
            ones = constp.tile([128, 1], bf16, tag="ones")
            nc.gpsimd.memset(ones, 1.0)
            csc_sb = constp.tile([128, TQ], f32, tag="csc")
            nc.sync.dma_start(csc_sb[:], csc[:])
            css_sb = constp.tile([128, TQ], f32, tag="css")
            nc.sync.dma_start(css_sb[:], css[:])

            # x_mid^T lives across attention + MLP
            xmT = pp.tile([128, NT, TQ], f32, tag="xmT")

            def norm_scale_row(ssq_ps, scale, bias, tag):
                """[1,TQ] psum sum-of-squares -> broadcast [128,TQ] f32 scale."""
                sr = wpool.tile([1, TQ], f32, tag="srow")
                nc.scalar.activation(sr[:], ssq_ps[:], Act.Sqrt, bias=bias, scale=scale)
                si = wpool.tile([1, TQ], f32, tag="srow")
                nc.vector.reciprocal(si[:], sr[:])
                sb = wpool.tile([128, TQ], f32, tag=tag)
                nc.gpsimd.partition_broadcast(sb[:], si[:])
                return sb

            def load_xT(i):
                xin = wpool.tile([128, TQ], f32, tag="xin")
                nc.sync.dma_start(xin[:], xT[128 * i:128 * (i + 1), :])
                return xin

            def rope(ps):
                """psum [128,TQ] f32 -> rope'd bf16 sbuf tile."""
                raw = wpool.tile([128, TQ], bf16, tag="rraw", bufs=4)
                nc.scalar.copy(raw[:], ps[:])
                sw = wpool.tile([128, TQ], bf16, tag="rsw", bufs=2)
                nc.sync.dma_start(sw[0:64, :], raw[64:128, :])
                nc.sync.dma_start(sw[64:128, :], raw[0:64, :])
                rr = wpool.tile([128, TQ], bf16, tag="rr", bufs=8)
                nc.vector.tensor_tensor(rr[:], raw[:], csc_sb[:], Alu.mult)
                t2 = wpool.tile([128, TQ], bf16, tag="rt2", bufs=2)
                nc.vector.tensor_tensor(t2[:], sw[:], css_sb[:], Alu.mult)
                nc.vector.tensor_tensor(rr[:], rr[:], t2[:], Alu.add)
                return rr

            def sumsq(rr):
                sq = wpool.tile([128, TQ], bf16, tag="xsq", bufs=8)
                nc.vector.tensor_tensor(sq[:], rr[:], rr[:], Alu.mult)
                return sq

            with tc.tile_pool(name="attn", bufs=1) as ap_:
                mask_sb = ap_.tile([128, NT, TQ], bf16, tag="mask_sb")
                nc.sync.dma_start(mask_sb[:], mask.rearrange("(g p) t -> p g t", p=128))
                qs_sb = ap_.tile([128, NH, TQ], bf16, tag="qs_sb")
                hT = ap_.tile([128, NT, TQ], bf16, tag="hT")
                vloc = ap_.tile([128, 4, TQ], bf16, tag="vloc")

                with tc.tile_pool(name="ps1", bufs=1, space="PSUM") as ps1:
                    # ---- P0: pre-attention rmsnorm (feature-major) ----
                    ssq_ps = ps1.tile([1, TQ], f32, tag="row", bufs=3)
                    for i in range(NT):
                        xin = load_xT(i)
                        xsq = wpool.tile([128, TQ], bf16, tag="xsq", bufs=8)
                        nc.vector.tensor_tensor(xsq[:], xin[:], xin[:], Alu.mult)
                        nc.tensor.matmul(ssq_ps[:], lhsT=ones[:], rhs=xsq[:],
                                         start=(i == 0), stop=(i == NT - 1))
                    s1b = norm_scale_row(ssq_ps, 1.0 / C, EPS, "sbcast")
                    for i in range(NT):
                        xin = load_xT(i)
                        nc.vector.tensor_tensor(hT[:, i], xin[:], s1b[:], Alu.mult)

                    # ---- K heads first: project + rope + k-norm -> cc_in ----
                    kps = [ps1.tile([128, TQ], f32, tag="qkv", bufs=4,
                                    name=f"kps_{_k}") for _k in range(4)]
                    for i in range(NT):
                        wk_sb = wsp.tile([128, TQ], bf16, tag="wq_sb")
                        nc.sync.dma_start(wk_sb[:], wk[128 * i:128 * (i + 1), :])
                        for k in range(4):
                            nc.tensor.matmul(kps[k][:],
                                             lhsT=wk_sb[:, 128 * k:128 * (k + 1)],
                                             rhs=hT[:, i],
                                             start=(i == 0), stop=(i == NT - 1))
                    for kh in range(4):
                        rr = rope(kps[kh])
                        sq = sumsq(rr)
                        sps = ps1.tile([1, TQ], f32, tag="row", bufs=3)
                        nc.tensor.matmul(sps[:], lhsT=ones[:], rhs=sq[:],
                                         start=True, stop=True)
                        sb = norm_scale_row(sps, 1.0 / HD, EPS, "sbcast")
                        kt = wpool.tile([128, TQ], bf16, tag="ktile")
                        nc.vector.tensor_tensor(kt[:], rr[:], sb[:], Alu.mult)
                        nc.sync.dma_start(ck_in[128 * kh:128 * (kh + 1), :], kt[:])

                    nc.gpsimd.collective_compute(
                        "AllGather", Alu.bypass,
                        replica_groups=[[0, 1, 2, 3], [4, 5, 6, 7]],
                        ins=[ck_in[:]], outs=[ck_out[:]])

                    # ---- V heads: project + transpose to token-major -> cv_in ----
                    vps = [ps1.tile([128, TQ], f32, tag="qkv", bufs=4,
                                    name=f"vps_{_k}") for _k in range(4)]
                    for i in range(NT):
                        wv_sb = wsp.tile([128, TQ], bf16, tag="wq_sb")
                        nc.sync.dma_start(wv_sb[:], wv[128 * i:128 * (i + 1), :])
                        for k in range(4):
                            nc.tensor.matmul(vps[k][:],
                                             lhsT=wv_sb[:, 128 * k:128 * (k + 1)],
                                             rhs=hT[:, i],
                                             start=(i == 0), stop=(i == NT - 1))
                    for kh in range(4):
                        vb = wpool.tile([128, TQ], bf16, tag="ktile")
                        nc.scalar.copy(vb[:], vps[kh][:])
                        for j in range(4):
                            tps = ps1.tile([128, 128], bf16, tag="tr", bufs=1)
                            nc.tensor.transpose(tps[:], vb[:, 128 * j:128 * (j + 1)],
                                                ident[:])
                            nc.vector.tensor_copy(
                                out=vloc[:, j, 128 * kh:128 * (kh + 1)], in_=tps[:])
                    for j in range(4):
                        nc.sync.dma_start(
                            cv_in[128 * j:128 * (j + 1), :], vloc[:, j, :])
                    nc.gpsimd.collective_compute(
                        "AllGather", Alu.bypass,
                        replica_groups=[[0, 1, 2, 3], [4, 5, 6, 7]],
                        ins=[cv_in[:]], outs=[cv_out[:]])

                    # ---- Q heads: project + rope + deferred q-norm ----
                    # ssq matmuls of group g are emitted after group g+1's
                    # projection matmuls so the PE never waits on the DVE chain
                    pending = None

                    def finish_q(pend):
                        hg, rrs, sqs = pend
                        for k in range(4):
                            h = 4 * hg + k
                            sps = ps1.tile([1, TQ], f32, tag="row", bufs=3)
                            nc.tensor.matmul(sps[:], lhsT=ones[:], rhs=sqs[k][:],
                                             start=True, stop=True)
                            sb = norm_scale_row(sps, 1.0, HD * EPS, "sbcast")
                            nc.vector.tensor_tensor(qs_sb[:, h], rrs[k][:], sb[:],
                                                    Alu.mult)

                    for hg in range(4):
                        qps = [ps1.tile([128, TQ], f32, tag="qkv", bufs=4,
                                        name=f"qps{hg}_{_k}") for _k in range(4)]
                        for i in range(NT):
                            wq_sb = wsp.tile([128, TQ], bf16, tag="wq_sb")
                            nc.sync.dma_start(wq_sb[:], wq[128 * i:128 * (i + 1),
                                                           TQ * hg:TQ * (hg + 1)])
                            for k in range(4):
                                nc.tensor.matmul(qps[k][:],
                                                 lhsT=wq_sb[:, 128 * k:128 * (k + 1)],
                                                 rhs=hT[:, i],
                                                 start=(i == 0), stop=(i == NT - 1))
                        if pending is not None:
                            finish_q(pending)
                        rrs = [rope(qps[k]) for k in range(4)]
                        sqs = [sumsq(rrs[k]) for k in range(4)]
                        pending = (hg, rrs, sqs)
                    finish_q(pending)

                # ---- load gathered K/V ----
                k_sb = ap_.tile([128, 16, TQ], bf16, tag="k_sb")   # (kh, g)
                v_sb = ap_.tile([128, 16, TQ], bf16, tag="v_sb")   # (g, j)
                for g in range(4):
                    for kh in range(4):
                        nc.sync.dma_start(
                            k_sb[:, 4 * kh + g],
                            ck_out[512 * g + 128 * kh:512 * g + 128 * (kh + 1), :])
                    for j in range(4):
                        nc.sync.dma_start(
                            v_sb[:, 4 * g + j],
                            cv_out[512 * g + 128 * j:512 * g + 128 * (j + 1), :])

                # ---- attention, 4 sibling q-heads per kv head together ----
                yT = ap_.tile([128, NH, TQ], bf16, tag="hT")
                with tc.tile_pool(name="ps2", bufs=1, space="PSUM") as ps2:
                    for kh in range(NKV):
                      for pr in range(2):
                        hs = [4 * kh + 2 * pr + k for k in range(2)]
                        den2 = ps2.tile([33, TQ], f32, tag="den", bufs=1)
                        y_ps = [ps2.tile([128, TQ], f32, tag="y", bufs=4,
                                         name=f"y{kh}{pr}_{_k}") for _k in range(2)]
                        for m in range(16):
                            g, mm = divmod(m, 4)
                            for k in range(2):
                                sc_ps = ps2.tile([128, TQ], f32, tag="sc", bufs=3)
                                nc.tensor.matmul(
                                    sc_ps[:],
                                    lhsT=k_sb[:, 4 * kh + g, 128 * mm:128 * (mm + 1)],
                                    rhs=qs_sb[:, hs[k]], start=True, stop=True)
                                p_bf = wpool.tile([128, TQ], bf16, tag="p_bf",
                                                  bufs=6)
                                nc.scalar.activation(p_bf[:], sc_ps[:], Act.Exp)
                                nc.vector.tensor_tensor(p_bf[:], p_bf[:],
                                                        mask_sb[:, m], Alu.mult)
                                nc.tensor.matmul(den2[32 * k:32 * k + 1, :],
                                                 lhsT=ones[:], rhs=p_bf[:],
                                                 start=(m == 0), stop=(m == 15))
                                nc.tensor.matmul(
                                    y_ps[k][:],
                                    lhsT=v_sb[:, m, 128 * kh:128 * (kh + 1)],
                                    rhs=p_bf[:],
                                    start=(m == 0), stop=(m == 15))
                        for k in range(2):
                            yraw = wpool.tile([128, TQ], f32, tag="yraw", bufs=3)
                            nc.scalar.copy(yraw[:], y_ps[k][:])
                            dr = wpool.tile([1, TQ], f32, tag="srow")
                            nc.vector.reciprocal(dr[:], den2[32 * k:32 * k + 1, :])
                            db = wpool.tile([128, TQ], f32, tag="sbcast")
                            nc.gpsimd.partition_broadcast(db[:], dr[:])
                            nc.vector.tensor_tensor(yT[:, hs[k]], yraw[:], db[:],
                                                    Alu.mult)

                    # ---- wo projection + residual (feature-major xmT) ----
                    for n4 in range(4):
                        att_ps = [ps2.tile([128, TQ], f32, tag="y", bufs=4,
                                           name=f"att{n4}_{_k}") for _k in range(4)]
                        for h in range(NH):
                            wo_sb = wsp.tile([128, TQ], bf16, tag="wo_sb")
                            nc.sync.dma_start(wo_sb[:], wo[128 * h:128 * (h + 1),
                                                           TQ * n4:TQ * (n4 + 1)])
                            for k in range(4):
                                nc.tensor.matmul(att_ps[k][:],
                                                 lhsT=wo_sb[:, 128 * k:128 * (k + 1)],
                                                 rhs=yT[:, h],
                                                 start=(h == 0), stop=(h == NH - 1))
                        for k in range(4):
                            n = 4 * n4 + k
                            xin = load_xT(n)
                            nc.vector.tensor_tensor(xmT[:, n], att_ps[k][:],
                                                    xin[:], Alu.add)
            # attn pool closed

            # ---- MLP ----
            with tc.tile_pool(name="mlp", bufs=1) as mp:
                h2T = mp.tile([128, NT, TQ], bf16, tag="h2T")
                a_sb = mp.tile([128, NF, TQ], bf16, tag="a_sb")
                xm_tm = mp.tile([128, 4, C], bf16, tag="xm_tm")  # token-major x_mid

                with tc.tile_pool(name="ps3", bufs=1, space="PSUM") as ps3:
                    ssq2 = ps3.tile([1, TQ], f32, tag="row", bufs=2)
                    _dummy = 0
                    for i in range(NT):
                        xsq = wpool.tile([128, TQ], bf16, tag="xsq", bufs=8)
                        nc.vector.tensor_tensor(xsq[:], xmT[:, i], xmT[:, i],
                                                Alu.mult)
                        nc.tensor.matmul(ssq2[:], lhsT=ones[:], rhs=xsq[:],
                                         start=(i == 0), stop=(i == NT - 1))
                    s2b = norm_scale_row(ssq2, 1.0 / C, EPS, "sbcast")
                    for i in range(NT):
                        nc.vector.tensor_tensor(h2T[:, i], xmT[:, i], s2b[:],
                                                Alu.mult)

                    # transpose xmT -> token-major for the final residual
                    for i in range(NT):
                        for j in range(4):
                            tp2 = ps3.tile([128, 128], f32, tag="mm", bufs=6)
                            nc.tensor.transpose(tp2[:],
                                                xmT[:, i, 128 * j:128 * (j + 1)],
                                                ident_f[:])
                            nc.vector.tensor_copy(
                                out=xm_tm[:, j, 128 * i:128 * (i + 1)], in_=tp2[:])

                # fc + relu^2 (feature-major a)
                with tc.tile_pool(name="ps3b", bufs=1, space="PSUM") as ps3b:
                    for jc in range(16):
                        f_ps = [ps3b.tile([128, TQ], f32, tag="mm", bufs=8,
                                          name=f"fps{jc}_{_k}") for _k in range(4)]
                        for i in range(NT):
                            wfc_sb = wsp.tile([128, TQ], bf16, tag="wfc_sb")
                            nc.sync.dma_start(wfc_sb[:], wfc[128 * i:128 * (i + 1),
                                                            TQ * jc:TQ * (jc + 1)])
                            for jf in range(4):
                                nc.tensor.matmul(
                                    f_ps[jf][:],
                                    lhsT=wfc_sb[:, 128 * jf:128 * (jf + 1)],
                                    rhs=h2T[:, i],
                                    start=(i == 0), stop=(i == NT - 1))
                        for jf in range(4):
                            f = 4 * jc + jf
                            r_bf = wpool.tile([128, TQ], bf16, tag="r_bf")
                            nc.scalar.activation(r_bf[:], f_ps[jf][:], Act.Relu)
                            nc.vector.tensor_tensor(a_sb[:, f], r_bf[:], r_bf[:],
                                                    Alu.mult)

                # proj: lhsT = a tile (1 LDW : 2 MMs), token-major output
                with tc.tile_pool(name="ps4", bufs=1, space="PSUM") as ps4:
                    for n2 in range(2):
                        o_ps = [ps4.tile([128, TQ], f32, tag="o", bufs=8,
                                         name=f"ops{n2}_{_k}") for _k in range(8)]
                        for f in range(NF):
                            wp0 = wsp.tile([128, TQ], bf16, tag="wp_sb")
                            nc.sync.dma_start(
                                wp0[:], wproj[128 * f:128 * (f + 1),
                                              1024 * n2:1024 * n2 + 512])
                            wp1 = wsp.tile([128, TQ], bf16, tag="wp_sb")
                            nc.sync.dma_start(
                                wp1[:], wproj[128 * f:128 * (f + 1),
                                              1024 * n2 + 512:1024 * (n2 + 1)])
                            for tj in range(4):
                                nc.tensor.matmul(
                                    o_ps[2 * tj][:],
                                    lhsT=a_sb[:, f, 128 * tj:128 * (tj + 1)],
                                    rhs=wp0[:],
                                    start=(f == 0), stop=(f == NF - 1))
                                nc.tensor.matmul(
                                    o_ps[2 * tj + 1][:],
                                    lhsT=a_sb[:, f, 128 * tj:128 * (tj + 1)],
                                    rhs=wp1[:],
                                    start=(f == 0), stop=(f == NF - 1))
                        for tj in range(4):
                            for half in range(2):
                                cstart = 1024 * n2 + 512 * half
                                ov = wpool.tile([128, TQ], f32, tag="yraw")
                                nc.vector.tensor_tensor(
                                    ov[:], o_ps[2 * tj + half][:],
                                    xm_tm[:, tj, cstart:cstart + 512], Alu.add)
                                nc.sync.dma_start(
                                    out_tm[128 * tj:128 * (tj + 1),
                                           cstart:cstart + 512], ov[:])

    nc.compile()
    return nc


def _make_in_maps(x, cos, sin, weights_b):
    import ml_dtypes
    bf = ml_dtypes.bfloat16
    cosT = cos[0, :, 0, :].T  # [64, T]
    sinT = sin[0, :, 0, :].T
    in_maps = []
    for c in range(NCORES):
        b, r = divmod(c, 4)
        sl = slice(TQ * r, TQ * (r + 1))
        qpos = np.arange(TQ * r, TQ * (r + 1))
        m = {
            "xT": np.ascontiguousarray(x[b, sl, :].T),
            "csc": np.ascontiguousarray(
                np.concatenate([cosT[:, sl], cosT[:, sl]], axis=0)),
            "css": np.ascontiguousarray(
                np.concatenate([sinT[:, sl], -sinT[:, sl]], axis=0)),
            "mask": (np.arange(T)[:, None] <= qpos[None, :]).astype(bf),
        }
        m.update(weights_b)
        in_maps.append(m)
    return in_maps


def kernel(x, cos, sin, wq, wk, wv, wo, w_fc, w_proj):
    global _CACHE
    import ml_dtypes
    from concourse.bass_utils import run_bass_kernel_spmd

    bf = ml_dtypes.bfloat16
    x = np.asarray(x, np.float32)
    cos = np.asarray(cos, np.float32)
    sin = np.asarray(sin, np.float32)
    weights_b = {
        "wq": np.asarray(wq, np.float32).astype(bf),
        "wk": np.asarray(wk, np.float32).astype(bf),
        "wv": np.asarray(wv, np.float32).astype(bf),
        "wo": np.asarray(wo, np.float32).astype(bf),
        "wfc": np.asarray(w_fc, np.float32).astype(bf),
        "wproj": np.asarray(w_proj, np.float32).astype(bf),
    }

    if _CACHE is None:
        _CACHE = _build()
    nc = _CACHE

    in_maps = _make_in_maps(x, cos, sin, weights_b)
    res = run_bass_kernel_spmd(nc, in_maps, list(range(NCORES)))
    out = np.empty((B, T, C), np.float32)
    for c in range(NCORES):
        b, r = divmod(c, 4)
        out[b, TQ * r:TQ * (r + 1), :] = res.results[c]["out"]
    return out


# revision 12
# speedup vs baseline: 1.0169x; 1.0169x over previous
"""Trainium2 Bass kernel for nn_Block_32762010534337 (dense transformer block).

Strategy: sequence-parallel over 8 cores. Core c owns 512 tokens (batch c//4,
token chunk c%4). Each core computes rmsnorm -> K/V projections (+rope, k-norm)
for its own tokens, AllGathers K/V within its batch group of 4 cores (overlapped
with the Q projections), then runs causal attention + wo + MLP (relu^2) for its
512 tokens with fully replicated bf16 weights. Activations stay feature-major
([channel, token]); the host transposes per-core inputs and the final residual
path switches to token-major via 64 PE transposes.
"""
import sys
import os

if "/opt/trn_rl_repo" not in sys.path:
    sys.path.insert(0, "/opt/trn_rl_repo")

import numpy as np

B, T, C = 2, 2048, 2048
NH, NKV, HD = 16, 4, 128
DFF = 4 * C
TQ = 512          # tokens per core
NT = C // 128     # 16 feature tiles
NF = DFF // 128   # 64 ff tiles
EPS = 1.1920929e-07
NCORES = 8

_CACHE = None


def _build():
    import concourse.bass as bass
    import concourse.tile as tile
    from concourse import mybir, bacc
    from concourse.masks import make_identity

    dt = mybir.dt
    f32, bf16 = dt.float32, dt.bfloat16
    Alu = mybir.AluOpType
    Act = mybir.ActivationFunctionType

    nc = bacc.Bacc("TRN2", target_bir_lowering=False, debug=False, num_devices=NCORES)

    for val in (EPS, HD * EPS):
        tns = nc.alloc_sbuf_tensor(f"const-f32-{val}", [128, 1], f32)
        nc.gpsimd.memset(tns.ap(), val)
        nc.const_aps.aps[(f32, val)] = tns.ap()
    nc.all_engine_barrier()

    xT = nc.declare_dram_parameter("xT", [C, TQ], f32, isOutput=False)
    csc = nc.declare_dram_parameter("csc", [128, TQ], f32, isOutput=False)
    css = nc.declare_dram_parameter("css", [128, TQ], f32, isOutput=False)
    mask = nc.declare_dram_parameter("mask", [T, TQ], bf16, isOutput=False)
    wq = nc.declare_dram_parameter("wq", [C, C], bf16, isOutput=False)
    wk = nc.declare_dram_parameter("wk", [C, NKV * HD], bf16, isOutput=False)
    wv = nc.declare_dram_parameter("wv", [C, NKV * HD], bf16, isOutput=False)
    wo = nc.declare_dram_parameter("wo", [C, C], bf16, isOutput=False)
    wfc = nc.declare_dram_parameter("wfc", [C, DFF], bf16, isOutput=False)
    wproj = nc.declare_dram_parameter("wproj", [DFF, C], bf16, isOutput=False)
    out_tm = nc.declare_dram_parameter("out", [TQ, C], f32, isOutput=True)

    ck_in = nc.dram_tensor("ck_in", [512, TQ], bf16)
    ck_out = nc.dram_tensor("ck_out", [2048, TQ], bf16)
    cv_in = nc.dram_tensor("cv_in", [512, TQ], bf16)
    cv_out = nc.dram_tensor("cv_out", [2048, TQ], bf16)

    with tile.TileContext(nc, num_cores=NCORES) as tc:
        with (
            tc.tile_pool(name="const", bufs=1) as constp,
            tc.tile_pool(name="persist", bufs=1) as pp,
            tc.tile_pool(name="work", bufs=3) as wpool,
            tc.tile_pool(name="wstream", bufs=3) as wsp,
        ):
            ident = constp.tile([128, 128], bf16, tag="ident")
            make_identity(nc, ident)
            ident_f = constp.tile([128, 128], f32, tag="identf")
            make_identity(nc, ident_f)
            ones = constp.tile([128, 1], bf16, tag="ones")
            nc.gpsimd.memset(ones, 1.0)
            csc_sb = constp.tile([128, TQ], f32, tag="csc")
            nc.sync.dma_start(csc_sb[:], csc[:])
            css_sb = constp.tile([128, TQ], f32, tag="css")
            nc.sync.dma_start(css_sb[:], css[:])

            # x_mid^T lives across attention + MLP
            xmT = pp.tile([128, NT, TQ], f32, tag="xmT")

            def norm_scale_row(ssq_ps, scale, bias, tag):
                """[1,TQ] psum sum-of-squares -> broadcast [128,TQ] f32 scale."""
                sr = wpool.tile([1, TQ], f32, tag="srow")
                nc.scalar.activation(sr[:], ssq_ps[:], Act.Sqrt, bias=bias, scale=scale)
                sb0 = wpool.tile([128, TQ], f32, tag=tag)
                nc.gpsimd.partition_broadcast(sb0[:], sr[:])
                sb = wpool.tile([128, TQ], f32, tag=tag)
                nc.vector.reciprocal(sb[:], sb0[:])
                return sb

            def load_xT(i):
                xin = wpool.tile([128, TQ], f32, tag="xin")
                nc.sync.dma_start(xin[:], xT[128 * i:128 * (i + 1), :])
                return xin

            def rope(ps):
                """psum [128,TQ] f32 -> rope'd bf16 sbuf tile."""
                raw = wpool.tile([128, TQ], bf16, tag="rraw", bufs=4)
                nc.scalar.copy(raw[:], ps[:])
                sw = wpool.tile([128, TQ], bf16, tag="rsw", bufs=2)
                nc.sync.dma_start(sw[0:64, :], raw[64:128, :])
                nc.sync.dma_start(sw[64:128, :], raw[0:64, :])
                rr = wpool.tile([128, TQ], bf16, tag="rr", bufs=8)
                nc.vector.tensor_tensor(rr[:], raw[:], csc_sb[:], Alu.mult)
                t2 = wpool.tile([128, TQ], bf16, tag="rt2", bufs=2)
                nc.vector.tensor_tensor(t2[:], sw[:], css_sb[:], Alu.mult)
                nc.vector.tensor_tensor(rr[:], rr[:], t2[:], Alu.add)
                return rr

            def sumsq(rr):
                sq = wpool.tile([128, TQ], bf16, tag="xsq", bufs=8)
                nc.vector.tensor_tensor(sq[:], rr[:], rr[:], Alu.mult)
                return sq

            with tc.tile_pool(name="attn", bufs=1) as ap_:
                mask_sb = ap_.tile([128, NT, TQ], bf16, tag="mask_sb")
                nc.sync.dma_start(mask_sb[:], mask.rearrange("(g p) t -> p g t", p=128))
                qs_sb = ap_.tile([128, NH, TQ], bf16, tag="qs_sb")
                hT = ap_.tile([128, NT, TQ], bf16, tag="hT")
                vloc = ap_.tile([128, 4, TQ], bf16, tag="vloc")

                with tc.tile_pool(name="ps1", bufs=1, space="PSUM") as ps1:
                    # ---- P0: pre-attention rmsnorm (feature-major) ----
                    ssq_ps = ps1.tile([1, TQ], f32, tag="row", bufs=3)
                    for i in range(NT):
                        xin = load_xT(i)
                        xsq = wpool.tile([128, TQ], bf16, tag="xsq", bufs=8)
                        nc.vector.tensor_tensor(xsq[:], xin[:], xin[:], Alu.mult)
                        nc.tensor.matmul(ssq_ps[:], lhsT=ones[:], rhs=xsq[:],
                                         start=(i == 0), stop=(i == NT - 1))
                    s1b = norm_scale_row(ssq_ps, 1.0 / C, EPS, "sbcast")
                    for i in range(NT):
                        xin = load_xT(i)
                        nc.vector.tensor_tensor(hT[:, i], xin[:], s1b[:], Alu.mult)

                    # ---- K heads first: project + rope + k-norm -> cc_in ----
                    kps = [ps1.tile([128, TQ], f32, tag="qkv", bufs=4,
                                    name=f"kps_{_k}") for _k in range(4)]
                    for i in range(NT):
                        wk_sb = wsp.tile([128, TQ], bf16, tag="wq_sb")
                        nc.sync.dma_start(wk_sb[:], wk[128 * i:128 * (i + 1), :])
                        for k in range(4):
                            nc.tensor.matmul(kps[k][:],
                                             lhsT=wk_sb[:, 128 * k:128 * (k + 1)],
                                             rhs=hT[:, i],
                                             start=(i == 0), stop=(i == NT - 1))
                    for kh in range(4):
                        rr = rope(kps[kh])
                        sq = sumsq(rr)
                        sps = ps1.tile([1, TQ], f32, tag="row", bufs=3)
                        nc.tensor.matmul(sps[:], lhsT=ones[:], rhs=sq[:],
                                         start=True, stop=True)
                        sb = norm_scale_row(sps, 1.0 / HD, EPS, "sbcast")
                        kt = wpool.tile([128, TQ], bf16, tag="ktile")
                        nc.vector.tensor_tensor(kt[:], rr[:], sb[:], Alu.mult)
                        nc.sync.dma_start(ck_in[128 * kh:128 * (kh + 1), :], kt[:])

                    nc.gpsimd.collective_compute(
                        "AllGather", Alu.bypass,
                        replica_groups=[[0, 1, 2, 3], [4, 5, 6, 7]],
                        ins=[ck_in[:]], outs=[ck_out[:]])

                    # ---- V heads: project + transpose to token-major -> cv_in ----
                    vps = [ps1.tile([128, TQ], f32, tag="qkv", bufs=4,
                                    name=f"vps_{_k}") for _k in range(4)]
                    for i in range(NT):
                        wv_sb = wsp.tile([128, TQ], bf16, tag="wq_sb")
                        nc.sync.dma_start(wv_sb[:], wv[128 * i:128 * (i + 1), :])
                        for k in range(4):
                            nc.tensor.matmul(vps[k][:],
                                             lhsT=wv_sb[:, 128 * k:128 * (k + 1)],
                                             rhs=hT[:, i],
                                             start=(i == 0), stop=(i == NT - 1))
                    for kh in range(4):
                        vb = wpool.tile([128, TQ], bf16, tag="ktile")
                        nc.scalar.copy(vb[:], vps[kh][:])
                        for j in range(4):
                            tps = ps1.tile([128, 128], bf16, tag="tr", bufs=1)
                            nc.tensor.transpose(tps[:], vb[:, 128 * j:128 * (j + 1)],
                                                ident[:])
                            nc.vector.tensor_copy(
                                out=vloc[:, j, 128 * kh:128 * (kh + 1)], in_=tps[:])
                    for j in range(4):
                        nc.sync.dma_start(
                            cv_in[128 * j:128 * (j + 1), :], vloc[:, j, :])
                    nc.gpsimd.collective_compute(
                        "AllGather", Alu.bypass,
                        replica_groups=[[0, 1, 2, 3], [4, 5, 6, 7]],
                        ins=[cv_in[:]], outs=[cv_out[:]])

                    # ---- Q heads: project + rope + deferred q-norm ----
                    # ssq matmuls of group g are emitted after group g+1's
                    # projection matmuls so the PE never waits on the DVE chain
                    pending = None

                    def finish_q(pend):
                        hg, rrs, sqs = pend
                        for k in range(4):
                            h = 4 * hg + k
                            sps = ps1.tile([1, TQ], f32, tag="row", bufs=3)
                            nc.tensor.matmul(sps[:], lhsT=ones[:], rhs=sqs[k][:],
                                             start=True, stop=True)
                            sb = norm_scale_row(sps, 1.0, HD * EPS, "sbcast")
                            nc.vector.tensor_tensor(qs_sb[:, h], rrs[k][:], sb[:],
                                                    Alu.mult)

                    for hg in range(4):
                        qps = [ps1.tile([128, TQ], f32, tag="qkv", bufs=4,
                                        name=f"qps{hg}_{_k}") for _k in range(4)]
                        for i in range(NT):
                            wq_sb = wsp.tile([128, TQ], bf16, tag="wq_sb")
                            nc.sync.dma_start(wq_sb[:], wq[128 * i:128 * (i + 1),
                                                           TQ * hg:TQ * (hg + 1)])
                            for k in range(4):
                                nc.tensor.matmul(qps[k][:],
                                                 lhsT=wq_sb[:, 128 * k:128 * (k + 1)],
                                                 rhs=hT[:, i],
                                                 start=(i == 0), stop=(i == NT - 1))
                        if pending is not None:
                            finish_q(pending)
                        rrs = [rope(qps[k]) for k in range(4)]
                        sqs = [sumsq(rrs[k]) for k in range(4)]
                        pending = (hg, rrs, sqs)
                    finish_q(pending)

                # ---- load gathered K/V ----
                k_sb = ap_.tile([128, 16, TQ], bf16, tag="k_sb")   # (kh, g)
                v_sb = ap_.tile([128, 16, TQ], bf16, tag="v_sb")   # (g, j)
                for g in range(4):
                    for kh in range(4):
                        nc.sync.dma_start(
                            k_sb[:, 4 * kh + g],
                            ck_out[512 * g + 128 * kh:512 * g + 128 * (kh + 1), :])
                    for j in range(4):
                        nc.sync.dma_start(
                            v_sb[:, 4 * g + j],
                            cv_out[512 * g + 128 * j:512 * g + 128 * (j + 1), :])

                # ---- attention, 4 sibling q-heads per kv head together ----
                yT = ap_.tile([128, NH, TQ], bf16, tag="hT")
                with tc.tile_pool(name="ps2", bufs=1, space="PSUM") as ps2:
                    for kh in range(NKV):
                      for pr in range(2):
                        hs = [4 * kh + 2 * pr + k for k in range(2)]
                        den2 = ps2.tile([33, TQ], f32, tag="den", bufs=1)
                        y_ps = [ps2.tile([128, TQ], f32, tag="y", bufs=4,
                                         name=f"y{kh}{pr}_{_k}") for _k in range(2)]
                        for m in range(16):
                            g, mm = divmod(m, 4)
                            for k in range(2):
                                sc_ps = ps2.tile([128, TQ], f32, tag="sc", bufs=3)
                                nc.tensor.matmul(
                                    sc_ps[:],
                                    lhsT=k_sb[:, 4 * kh + g, 128 * mm:128 * (mm + 1)],
                                    rhs=qs_sb[:, hs[k]], start=True, stop=True)
                                p_bf = wpool.tile([128, TQ], bf16, tag="p_bf",
                                                  bufs=6)
                                nc.scalar.activation(p_bf[:], sc_ps[:], Act.Exp)
                                nc.vector.tensor_tensor(p_bf[:], p_bf[:],
                                                        mask_sb[:, m], Alu.mult)
                                nc.tensor.matmul(den2[32 * k:32 * k + 1, :],
                                                 lhsT=ones[:], rhs=p_bf[:],
                                                 start=(m == 0), stop=(m == 15))
                                nc.tensor.matmul(
                                    y_ps[k][:],
                                    lhsT=v_sb[:, m, 128 * kh:128 * (kh + 1)],
                                    rhs=p_bf[:],
                                    start=(m == 0), stop=(m == 15))
                        for k in range(2):
                            yraw = wpool.tile([128, TQ], f32, tag="yraw", bufs=3)
                            nc.scalar.copy(yraw[:], y_ps[k][:])
                            dr = wpool.tile([1, TQ], f32, tag="srow")
                            nc.scalar.copy(dr[:], den2[32 * k:32 * k + 1, :])
                            db0 = wpool.tile([128, TQ], f32, tag="sbcast")
                            nc.gpsimd.partition_broadcast(db0[:], dr[:])
                            db = wpool.tile([128, TQ], f32, tag="sbcast")
                            nc.vector.reciprocal(db[:], db0[:])
                            nc.vector.tensor_tensor(yT[:, hs[k]], yraw[:], db[:],
                                                    Alu.mult)

                    # ---- wo projection + residual (feature-major xmT) ----
                    for n4 in range(4):
                        att_ps = [ps2.tile([128, TQ], f32, tag="y", bufs=4,
                                           name=f"att{n4}_{_k}") for _k in range(4)]
                        for h in range(NH):
                            wo_sb = wsp.tile([128, TQ], bf16, tag="wo_sb")
                            nc.sync.dma_start(wo_sb[:], wo[128 * h:128 * (h + 1),
                                                           TQ * n4:TQ * (n4 + 1)])
                            for k in range(4):
                                nc.tensor.matmul(att_ps[k][:],
                                                 lhsT=wo_sb[:, 128 * k:128 * (k + 1)],
                                                 rhs=yT[:, h],
                                                 start=(h == 0), stop=(h == NH - 1))
                        for k in range(4):
                            n = 4 * n4 + k
                            xin = load_xT(n)
                            nc.vector.tensor_tensor(xmT[:, n], att_ps[k][:],
                                                    xin[:], Alu.add)
            # attn pool closed

            # ---- MLP ----
            with tc.tile_pool(name="mlp", bufs=1) as mp:
                h2T = mp.tile([128, NT, TQ], bf16, tag="h2T")
                a_sb = mp.tile([128, NF, TQ], bf16, tag="a_sb")
                xm_tm = mp.tile([128, 4, C], bf16, tag="xm_tm")  # token-major x_mid

                with tc.tile_pool(name="ps3", bufs=1, space="PSUM") as ps3:
                    ssq2 = ps3.tile([1, TQ], f32, tag="row", bufs=2)
                    _dummy = 0
                    for i in range(NT):
                        xsq = wpool.tile([128, TQ], bf16, tag="xsq", bufs=8)
                        nc.vector.tensor_tensor(xsq[:], xmT[:, i], xmT[:, i],
                                                Alu.mult)
                        nc.tensor.matmul(ssq2[:], lhsT=ones[:], rhs=xsq[:],
                                         start=(i == 0), stop=(i == NT - 1))
                    s2b = norm_scale_row(ssq2, 1.0 / C, EPS, "sbcast")
                    for i in range(NT):
                        nc.vector.tensor_tensor(h2T[:, i], xmT[:, i], s2b[:],
                                                Alu.mult)

                    # transpose xmT -> token-major for the final residual
                    for i in range(NT):
                        for j in range(4):
                            tp2 = ps3.tile([128, 128], f32, tag="mm", bufs=6)
                            nc.tensor.transpose(tp2[:],
                                                xmT[:, i, 128 * j:128 * (j + 1)],
                                                ident_f[:])
                            nc.vector.tensor_copy(
                                out=xm_tm[:, j, 128 * i:128 * (i + 1)], in_=tp2[:])

                # fc + relu^2 (feature-major a)
                with tc.tile_pool(name="ps3b", bufs=1, space="PSUM") as ps3b:
                    for jc in range(16):
                        f_ps = [ps3b.tile([128, TQ], f32, tag="mm", bufs=8,
                                          name=f"fps{jc}_{_k}") for _k in range(4)]
                        for i in range(NT):
                            wfc_sb = wsp.tile([128, TQ], bf16, tag="wfc_sb")
                            nc.sync.dma_start(wfc_sb[:], wfc[128 * i:128 * (i + 1),
                                                            TQ * jc:TQ * (jc + 1)])
                            for jf in range(4):
                                nc.tensor.matmul(
                                    f_ps[jf][:],
                                    lhsT=wfc_sb[:, 128 * jf:128 * (jf + 1)],
                                    rhs=h2T[:, i],
                                    start=(i == 0), stop=(i == NT - 1))
                        for jf in range(4):
                            f = 4 * jc + jf
                            r_bf = wpool.tile([128, TQ], bf16, tag="r_bf")
                            nc.scalar.activation(r_bf[:], f_ps[jf][:], Act.Relu)
                            nc.vector.tensor_tensor(a_sb[:, f], r_bf[:], r_bf[:],
                                                    Alu.mult)

                # proj: lhsT = a tile (1 LDW : 2 MMs), token-major output
                with tc.tile_pool(name="ps4", bufs=1, space="PSUM") as ps4:
                    for n2 in range(2):
                        o_ps = [ps4.tile([128, TQ], f32, tag="o", bufs=8,
                                         name=f"ops{n2}_{_k}") for _k in range(8)]
                        for f in range(NF):
                            wp0 = wsp.tile([128, TQ], bf16, tag="wp_sb")
                            nc.sync.dma_start(
                                wp0[:], wproj[128 * f:128 * (f + 1),
                                              1024 * n2:1024 * n2 + 512])
                            wp1 = wsp.tile([128, TQ], bf16, tag="wp_sb")
                            nc.sync.dma_start(
                                wp1[:], wproj[128 * f:128 * (f + 1),
                                              1024 * n2 + 512:1024 * (n2 + 1)])
                            for tj in range(4):
                                nc.tensor.matmul(
                                    o_ps[2 * tj][:],
                                    lhsT=a_sb[:, f, 128 * tj:128 * (tj + 1)],
                                    rhs=wp0[:],
                                    start=(f == 0), stop=(f == NF - 1))
                                nc.tensor.matmul(
                                    o_ps[2 * tj + 1][:],
                                    lhsT=a_sb[:, f, 128 * tj:128 * (tj + 1)],
                                    rhs=wp1[:],
                                    start=(f == 0), stop=(f == NF - 1))
                        for tj in range(4):
                            for half in range(2):
                                cstart = 1024 * n2 + 512 * half
                                ov = wpool.tile([128, TQ], f32, tag="yraw")
                                nc.vector.tensor_tensor(
                                    ov[:], o_ps[2 * tj + half][:],
                                    xm_tm[:, tj, cstart:cstart + 512], Alu.add)
                                nc.sync.dma_start(
                                    out_tm[128 * tj:128 * (tj + 1),
                                           cstart:cstart + 512], ov[:])

    nc.compile()
    return nc


def _make_in_maps(x, cos, sin, weights_b):
    import ml_dtypes
    bf = ml_dtypes.bfloat16
    cosT = cos[0, :, 0, :].T  # [64, T]
    sinT = sin[0, :, 0, :].T
    in_maps = []
    for c in range(NCORES):
        b, r = divmod(c, 4)
        sl = slice(TQ * r, TQ * (r + 1))
        qpos = np.arange(TQ * r, TQ * (r + 1))
        m = {
            "xT": np.ascontiguousarray(x[b, sl, :].T),
            "csc": np.ascontiguousarray(
                np.concatenate([cosT[:, sl], cosT[:, sl]], axis=0)),
            "css": np.ascontiguousarray(
                np.concatenate([sinT[:, sl], -sinT[:, sl]], axis=0)),
            "mask": (np.arange(T)[:, None] <= qpos[None, :]).astype(bf),
        }
        m.update(weights_b)
        in_maps.append(m)
    return in_maps


def kernel(x, cos, sin, wq, wk, wv, wo, w_fc, w_proj):
    global _CACHE
    import ml_dtypes
    from concourse.bass_utils import run_bass_kernel_spmd

    bf = ml_dtypes.bfloat16
    x = np.asarray(x, np.float32)
    cos = np.asarray(cos, np.float32)
    sin = np.asarray(sin, np.float32)
    weights_b = {
        "wq": np.asarray(wq, np.float32).astype(bf),
        "wk": np.asarray(wk, np.float32).astype(bf),
        "wv": np.asarray(wv, np.float32).astype(bf),
        "wo": np.asarray(wo, np.float32).astype(bf),
        "wfc": np.asarray(w_fc, np.float32).astype(bf),
        "wproj": np.asarray(w_proj, np.float32).astype(bf),
    }

    if _CACHE is None:
        _CACHE = _build()
    nc = _CACHE

    in_maps = _make_in_maps(x, cos, sin, weights_b)
    res = run_bass_kernel_spmd(nc, in_maps, list(range(NCORES)))
    out = np.empty((B, T, C), np.float32)
    for c in range(NCORES):
        b, r = divmod(c, 4)
        out[b, TQ * r:TQ * (r + 1), :] = res.results[c]["out"]
    return out


# revision 17
# speedup vs baseline: 1.1014x; 1.0832x over previous
"""Trainium2 Bass kernel for nn_Block_32762010534337 (dense transformer block).

Strategy: sequence-parallel over 8 cores. Core c owns 512 tokens (batch c//4,
token chunk c%4). Each core computes rmsnorm -> K/V projections (+rope, k-norm)
for its own tokens, AllGathers K/V within its batch group of 4 cores (overlapped
with the Q projections), then runs causal attention + wo + MLP (relu^2) for its
512 tokens with fully replicated bf16 weights. Activations stay feature-major
([channel, token]); the host transposes per-core inputs and the final residual
path switches to token-major via 64 PE transposes.
"""
import sys
import os

if "/opt/trn_rl_repo" not in sys.path:
    sys.path.insert(0, "/opt/trn_rl_repo")

import numpy as np

B, T, C = 2, 2048, 2048
NH, NKV, HD = 16, 4, 128
DFF = 4 * C
TQ = 512          # tokens per core
NT = C // 128     # 16 feature tiles
NF = DFF // 128   # 64 ff tiles
EPS = 1.1920929e-07
NCORES = 8

_CACHE = None


def _build():
    import concourse.bass as bass
    import concourse.tile as tile
    from concourse import mybir, bacc
    from concourse.masks import make_identity

    dt = mybir.dt
    f32, bf16 = dt.float32, dt.bfloat16
    Alu = mybir.AluOpType
    Act = mybir.ActivationFunctionType

    nc = bacc.Bacc("TRN2", target_bir_lowering=False, debug=False, num_devices=NCORES)

    for val in (EPS, HD * EPS):
        tns = nc.alloc_sbuf_tensor(f"const-f32-{val}", [128, 1], f32)
        nc.gpsimd.memset(tns.ap(), val)
        nc.const_aps.aps[(f32, val)] = tns.ap()
    nc.all_engine_barrier()

    xT = nc.declare_dram_parameter("xT", [C, TQ], f32, isOutput=False)
    csc = nc.declare_dram_parameter("csc", [128, TQ], f32, isOutput=False)
    css = nc.declare_dram_parameter("css", [128, TQ], f32, isOutput=False)
    mask = nc.declare_dram_parameter("mask", [T, TQ], bf16, isOutput=False)
    wq = nc.declare_dram_parameter("wq", [C, C], bf16, isOutput=False)
    wk = nc.declare_dram_parameter("wk", [C, NKV * HD], bf16, isOutput=False)
    wv = nc.declare_dram_parameter("wv", [C, NKV * HD], bf16, isOutput=False)
    wo = nc.declare_dram_parameter("wo", [C, C], bf16, isOutput=False)
    wfc = nc.declare_dram_parameter("wfc", [C, DFF], bf16, isOutput=False)
    wproj = nc.declare_dram_parameter("wproj", [DFF, C], bf16, isOutput=False)
    out_tm = nc.declare_dram_parameter("out", [TQ, C], f32, isOutput=True)

    ck_in = nc.dram_tensor("ck_in", [512, TQ], bf16)
    ck_out = nc.dram_tensor("ck_out", [2048, TQ], bf16)
    cv_in = nc.dram_tensor("cv_in", [512, TQ], bf16)
    cv_out = nc.dram_tensor("cv_out", [2048, TQ], bf16)

    with tile.TileContext(nc, num_cores=NCORES) as tc:
        with (
            tc.tile_pool(name="const", bufs=1) as constp,
            tc.tile_pool(name="persist", bufs=1) as pp,
            tc.tile_pool(name="work", bufs=3) as wpool,
            tc.tile_pool(name="wstream", bufs=3) as wsp,
        ):
            ident = constp.tile([128, 128], bf16, tag="ident")
            make_identity(nc, ident)
            ident_f = constp.tile([128, 128], f32, tag="identf")
            make_identity(nc, ident_f)
            ones = constp.tile([128, 1], bf16, tag="ones")
            nc.gpsimd.memset(ones, 1.0)
            csc_sb = constp.tile([128, TQ], f32, tag="csc")
            nc.sync.dma_start(csc_sb[:], csc[:])
            css_sb = constp.tile([128, TQ], f32, tag="css")
            nc.sync.dma_start(css_sb[:], css[:])

            # x_mid^T lives across attention + MLP
            xmT = pp.tile([128, NT, TQ], f32, tag="xmT")

            def norm_scale_row(ssq_ps, scale, bias, tag):
                """[1,TQ] psum sum-of-squares -> broadcast [128,TQ] f32 scale."""
                sr = wpool.tile([1, TQ], f32, tag="srow")
                nc.scalar.activation(sr[:], ssq_ps[:], Act.Sqrt, bias=bias, scale=scale)
                sb0 = wpool.tile([128, TQ], f32, tag=tag)
                nc.gpsimd.partition_broadcast(sb0[:], sr[:])
                sb = wpool.tile([128, TQ], f32, tag=tag)
                nc.vector.reciprocal(sb[:], sb0[:])
                return sb

            def load_xT(i, pool):
                xin = pool.tile([128, TQ], f32, tag="xin", bufs=3, name="xin")
                nc.sync.dma_start(xin[:], xT[128 * i:128 * (i + 1), :])
                return xin

            def rope(ps, pool):
                """psum [128,TQ] f32 -> rope'd bf16 sbuf tile."""
                raw = pool.tile([128, TQ], bf16, tag="rraw", bufs=4, name="rraw")
                nc.scalar.copy(raw[:], ps[:])
                sw = pool.tile([128, TQ], bf16, tag="rsw", bufs=2, name="rsw")
                nc.sync.dma_start(sw[0:64, :], raw[64:128, :])
                nc.sync.dma_start(sw[64:128, :], raw[0:64, :])
                rr = pool.tile([128, TQ], bf16, tag="rr", bufs=6, name="rr")
                nc.vector.tensor_tensor(rr[:], raw[:], csc_sb[:], Alu.mult)
                t2 = pool.tile([128, TQ], bf16, tag="rt2", bufs=2, name="rt2")
                nc.vector.tensor_tensor(t2[:], sw[:], css_sb[:], Alu.mult)
                nc.vector.tensor_tensor(rr[:], rr[:], t2[:], Alu.add)
                return rr

            def sumsq(rr, pool):
                sq = pool.tile([128, TQ], bf16, tag="rsq", bufs=6, name="rsq")
                nc.vector.tensor_tensor(sq[:], rr[:], rr[:], Alu.mult)
                return sq

            with tc.tile_pool(name="attn", bufs=1) as ap_:
                mask_sb = ap_.tile([128, NT, TQ], bf16, tag="mask_sb")
                nc.sync.dma_start(mask_sb[:], mask.rearrange("(g p) t -> p g t", p=128))
                qs_sb = ap_.tile([128, NH, TQ], bf16, tag="qs_sb")
                hT = ap_.tile([128, NT, TQ], bf16, tag="hT")
                vloc = ap_.tile([128, 4, TQ], bf16, tag="vloc")

                with tc.tile_pool(name="ps1", bufs=1, space="PSUM") as ps1:
                    # ---- P0: pre-attention rmsnorm (feature-major) ----
                    ssq_ps = ps1.tile([1, TQ], f32, tag="row", bufs=3)
                    for i in range(NT):
                        xin = load_xT(i, ap_)
                        xsq = wpool.tile([128, TQ], bf16, tag="xsq", bufs=8)
                        nc.vector.tensor_tensor(xsq[:], xin[:], xin[:], Alu.mult)
                        nc.tensor.matmul(ssq_ps[:], lhsT=ones[:], rhs=xsq[:],
                                         start=(i == 0), stop=(i == NT - 1))
                    s1b = norm_scale_row(ssq_ps, 1.0 / C, EPS, "sbcast")
                    for i in range(NT):
                        xin = load_xT(i, ap_)
                        nc.vector.tensor_tensor(hT[:, i], xin[:], s1b[:], Alu.mult)

                    # ---- K heads first: project + rope + k-norm -> cc_in ----
                    kps = [ps1.tile([128, TQ], f32, tag="qkv", bufs=4,
                                    name=f"kps_{_k}") for _k in range(4)]
                    for i in range(NT):
                        wk_sb = wsp.tile([128, TQ], bf16, tag="wq_sb")
                        nc.sync.dma_start(wk_sb[:], wk[128 * i:128 * (i + 1), :])
                        for k in range(4):
                            nc.tensor.matmul(kps[k][:],
                                             lhsT=wk_sb[:, 128 * k:128 * (k + 1)],
                                             rhs=hT[:, i],
                                             start=(i == 0), stop=(i == NT - 1))
                    for kh in range(4):
                        rr = rope(kps[kh], ap_)
                        sq = sumsq(rr, ap_)
                        sps = ps1.tile([1, TQ], f32, tag="row", bufs=3)
                        nc.tensor.matmul(sps[:], lhsT=ones[:], rhs=sq[:],
                                         start=True, stop=True)
                        sb = norm_scale_row(sps, 1.0 / HD, EPS, "sbcast")
                        kt = ap_.tile([128, TQ], bf16, tag="ktile", bufs=3, name="kt")
                        nc.vector.tensor_tensor(kt[:], rr[:], sb[:], Alu.mult)
                        nc.sync.dma_start(ck_in[128 * kh:128 * (kh + 1), :], kt[:])

                    nc.gpsimd.collective_compute(
                        "AllGather", Alu.bypass,
                        replica_groups=[[0, 1, 2, 3], [4, 5, 6, 7]],
                        ins=[ck_in[:]], outs=[ck_out[:]])

                    # ---- V heads: project + transpose to token-major -> cv_in ----
                    vps = [ps1.tile([128, TQ], f32, tag="qkv", bufs=4,
                                    name=f"vps_{_k}") for _k in range(4)]
                    for i in range(NT):
                        wv_sb = wsp.tile([128, TQ], bf16, tag="wq_sb")
                        nc.sync.dma_start(wv_sb[:], wv[128 * i:128 * (i + 1), :])
                        for k in range(4):
                            nc.tensor.matmul(vps[k][:],
                                             lhsT=wv_sb[:, 128 * k:128 * (k + 1)],
                                             rhs=hT[:, i],
                                             start=(i == 0), stop=(i == NT - 1))
                    for kh in range(4):
                        vb = ap_.tile([128, TQ], bf16, tag="ktile", bufs=3, name="vb")
                        nc.scalar.copy(vb[:], vps[kh][:])
                        for j in range(4):
                            tps = ps1.tile([128, 128], bf16, tag="tr", bufs=1)
                            nc.tensor.transpose(tps[:], vb[:, 128 * j:128 * (j + 1)],
                                                ident[:])
                            nc.vector.tensor_copy(
                                out=vloc[:, j, 128 * kh:128 * (kh + 1)], in_=tps[:])
                    for j in range(4):
                        nc.sync.dma_start(
                            cv_in[128 * j:128 * (j + 1), :], vloc[:, j, :])
                    nc.gpsimd.collective_compute(
                        "AllGather", Alu.bypass,
                        replica_groups=[[0, 1, 2, 3], [4, 5, 6, 7]],
                        ins=[cv_in[:]], outs=[cv_out[:]])

                    # ---- Q heads: project + rope + deferred q-norm ----
                    # ssq matmuls of group g are emitted after group g+1's
                    # projection matmuls so the PE never waits on the DVE chain
                    pending = None

                    def finish_q(pend):
                        hg, rrs, sqs = pend
                        for k in range(4):
                            h = 4 * hg + k
                            sps = ps1.tile([1, TQ], f32, tag="row", bufs=3)
                            nc.tensor.matmul(sps[:], lhsT=ones[:], rhs=sqs[k][:],
                                             start=True, stop=True)
                            sb = norm_scale_row(sps, 1.0, HD * EPS, "sbcast")
                            nc.vector.tensor_tensor(qs_sb[:, h], rrs[k][:], sb[:],
                                                    Alu.mult)

                    for hg in range(4):
                        qps = [ps1.tile([128, TQ], f32, tag="qkv", bufs=4,
                                        name=f"qps{hg}_{_k}") for _k in range(4)]
                        wts = []
                        for i in range(NT):
                            wq_sb = wsp.tile([128, TQ], bf16, tag="wtile", bufs=16,
                                             name=f"wq{hg}_{i}")
                            nc.sync.dma_start(wq_sb[:], wq[128 * i:128 * (i + 1),
                                                           TQ * hg:TQ * (hg + 1)])
                            wts.append(wq_sb)
                        rrs = []
                        sqs = []
                        for k in range(4):
                            for i in range(NT):
                                nc.tensor.matmul(qps[k][:],
                                                 lhsT=wts[i][:, 128 * k:128 * (k + 1)],
                                                 rhs=hT[:, i],
                                                 start=(i == 0), stop=(i == NT - 1))
                            rrs.append(rope(qps[k], ap_))
                            sqs.append(sumsq(rrs[k], ap_))
                        if pending is not None:
                            finish_q(pending)
                        pending = (hg, rrs, sqs)
                    finish_q(pending)

                # ---- load gathered K/V ----
                k_sb = ap_.tile([128, 16, TQ], bf16, tag="k_sb")   # (kh, g)
                v_sb = ap_.tile([128, 16, TQ], bf16, tag="v_sb")   # (g, j)
                for g in range(4):
                    for kh in range(4):
                        nc.sync.dma_start(
                            k_sb[:, 4 * kh + g],
                            ck_out[512 * g + 128 * kh:512 * g + 128 * (kh + 1), :])
                    for j in range(4):
                        nc.sync.dma_start(
                            v_sb[:, 4 * g + j],
                            cv_out[512 * g + 128 * j:512 * g + 128 * (j + 1), :])

                # ---- attention, 4 sibling q-heads per kv head together ----
                yT = ap_.tile([128, NH, TQ], bf16, tag="hT")
                with tc.tile_pool(name="ps2", bufs=1, space="PSUM") as ps2:
                    for kh in range(NKV):
                      for pr in range(2):
                        hs = [4 * kh + 2 * pr + k for k in range(2)]
                        den2 = ps2.tile([33, TQ], f32, tag="den", bufs=1)
                        y_ps = [ps2.tile([128, TQ], f32, tag="y", bufs=4,
                                         name=f"y{kh}{pr}_{_k}") for _k in range(2)]
                        fifo = []

                        def drain_one():
                            m0, k0, p0 = fifo.pop(0)
                            nc.tensor.matmul(den2[32 * k0:32 * k0 + 1, :],
                                             lhsT=ones[:], rhs=p0[:],
                                             start=(m0 == 0), stop=(m0 == 15))
                            nc.tensor.matmul(
                                y_ps[k0][:],
                                lhsT=v_sb[:, m0, 128 * kh:128 * (kh + 1)],
                                rhs=p0[:],
                                start=(m0 == 0), stop=(m0 == 15))

                        for m in range(16):
                            g, mm = divmod(m, 4)
                            for k in range(2):
                                sc_ps = ps2.tile([128, TQ], f32, tag="sc", bufs=3)
                                nc.tensor.matmul(
                                    sc_ps[:],
                                    lhsT=k_sb[:, 4 * kh + g, 128 * mm:128 * (mm + 1)],
                                    rhs=qs_sb[:, hs[k]], start=True, stop=True)
                                p_bf = ap_.tile([128, TQ], bf16, tag="p_bf",
                                                bufs=7, name="p_bf")
                                nc.scalar.activation(p_bf[:], sc_ps[:], Act.Exp)
                                nc.vector.tensor_tensor(p_bf[:], p_bf[:],
                                                        mask_sb[:, m], Alu.mult)
                                fifo.append((m, k, p_bf))
                                if len(fifo) > 4:
                                    drain_one()
                        while fifo:
                            drain_one()
                        for k in range(2):
                            yraw = wpool.tile([128, TQ], f32, tag="yraw", bufs=3)
                            nc.scalar.copy(yraw[:], y_ps[k][:])
                            dr = wpool.tile([1, TQ], f32, tag="srow")
                            nc.scalar.copy(dr[:], den2[32 * k:32 * k + 1, :])
                            db0 = wpool.tile([128, TQ], f32, tag="sbcast")
                            nc.gpsimd.partition_broadcast(db0[:], dr[:])
                            db = wpool.tile([128, TQ], f32, tag="sbcast")
                            nc.vector.reciprocal(db[:], db0[:])
                            nc.vector.tensor_tensor(yT[:, hs[k]], yraw[:], db[:],
                                                    Alu.mult)

                    # ---- wo projection + residual (feature-major xmT) ----
                    for n4 in range(4):
                        att_ps = [ps2.tile([128, TQ], f32, tag="y", bufs=4,
                                           name=f"att{n4}_{_k}") for _k in range(4)]
                        wts = []
                        for h in range(NH):
                            wo_sb = wsp.tile([128, TQ], bf16, tag="wtile", bufs=16,
                                             name=f"wo{n4}_{h}")
                            nc.sync.dma_start(wo_sb[:], wo[128 * h:128 * (h + 1),
                                                           TQ * n4:TQ * (n4 + 1)])
                            wts.append(wo_sb)
                        for k in range(4):
                            for h in range(NH):
                                nc.tensor.matmul(att_ps[k][:],
                                                 lhsT=wts[h][:, 128 * k:128 * (k + 1)],
                                                 rhs=yT[:, h],
                                                 start=(h == 0), stop=(h == NH - 1))
                            n = 4 * n4 + k
                            xin = load_xT(n, ap_)
                            nc.vector.tensor_tensor(xmT[:, n], att_ps[k][:],
                                                    xin[:], Alu.add)
            # attn pool closed

            # ---- MLP ----
            with tc.tile_pool(name="mlp", bufs=1) as mp:
                h2T = mp.tile([128, NT, TQ], bf16, tag="h2T")
                a_sb = mp.tile([128, NF, TQ], bf16, tag="a_sb")
                xm_tm = mp.tile([128, 4, C], bf16, tag="xm_tm")  # token-major x_mid

                with tc.tile_pool(name="ps3", bufs=1, space="PSUM") as ps3:
                    ssq2 = ps3.tile([1, TQ], f32, tag="row", bufs=2)
                    _dummy = 0
                    for i in range(NT):
                        xsq = wpool.tile([128, TQ], bf16, tag="xsq", bufs=8)
                        nc.vector.tensor_tensor(xsq[:], xmT[:, i], xmT[:, i],
                                                Alu.mult)
                        nc.tensor.matmul(ssq2[:], lhsT=ones[:], rhs=xsq[:],
                                         start=(i == 0), stop=(i == NT - 1))
                    s2b = norm_scale_row(ssq2, 1.0 / C, EPS, "sbcast")
                    for i in range(NT):
                        nc.vector.tensor_tensor(h2T[:, i], xmT[:, i], s2b[:],
                                                Alu.mult)

                    # transpose xmT -> token-major for the final residual
                    for i in range(NT):
                        for j in range(4):
                            tp2 = ps3.tile([128, 128], f32, tag="mm", bufs=6)
                            nc.tensor.transpose(tp2[:],
                                                xmT[:, i, 128 * j:128 * (j + 1)],
                                                ident_f[:])
                            nc.vector.tensor_copy(
                                out=xm_tm[:, j, 128 * i:128 * (i + 1)], in_=tp2[:])

                # fc + relu^2 (feature-major a)
                with tc.tile_pool(name="ps3b", bufs=1, space="PSUM") as ps3b:
                    for jc in range(16):
                        f_ps = [ps3b.tile([128, TQ], f32, tag="mm", bufs=8,
                                          name=f"fps{jc}_{_k}") for _k in range(4)]
                        wts = []
                        for i in range(NT):
                            wfc_sb = wsp.tile([128, TQ], bf16, tag="wtile", bufs=16,
                                              name=f"wfc{jc}_{i}")
                            nc.sync.dma_start(wfc_sb[:], wfc[128 * i:128 * (i + 1),
                                                            TQ * jc:TQ * (jc + 1)])
                            wts.append(wfc_sb)
                        for jf in range(4):
                            for i in range(NT):
                                nc.tensor.matmul(
                                    f_ps[jf][:],
                                    lhsT=wts[i][:, 128 * jf:128 * (jf + 1)],
                                    rhs=h2T[:, i],
                                    start=(i == 0), stop=(i == NT - 1))
                            f = 4 * jc + jf
                            r_bf = wpool.tile([128, TQ], bf16, tag="r_bf")
                            nc.scalar.activation(r_bf[:], f_ps[jf][:], Act.Relu)
                            nc.vector.tensor_tensor(a_sb[:, f], r_bf[:], r_bf[:],
                                                    Alu.mult)

                # proj: lhsT = a tile (1 LDW : 2 MMs), token-major output
                with tc.tile_pool(name="ps4", bufs=1, space="PSUM") as ps4:
                    for n2 in range(2):
                        o_ps = [ps4.tile([128, TQ], f32, tag="o", bufs=8,
                                         name=f"ops{n2}_{_k}") for _k in range(8)]
                        for f in range(NF):
                            wp0 = wsp.tile([128, TQ], bf16, tag="wp_sb")
                            nc.sync.dma_start(
                                wp0[:], wproj[128 * f:128 * (f + 1),
                                              1024 * n2:1024 * n2 + 512])
                            wp1 = wsp.tile([128, TQ], bf16, tag="wp_sb")
                            nc.sync.dma_start(
                                wp1[:], wproj[128 * f:128 * (f + 1),
                                              1024 * n2 + 512:1024 * (n2 + 1)])
                            for tj in range(4):
                                nc.tensor.matmul(
                                    o_ps[2 * tj][:],
                                    lhsT=a_sb[:, f, 128 * tj:128 * (tj + 1)],
                                    rhs=wp0[:],
                                    start=(f == 0), stop=(f == NF - 1))
                                nc.tensor.matmul(
                                    o_ps[2 * tj + 1][:],
                                    lhsT=a_sb[:, f, 128 * tj:128 * (tj + 1)],
                                    rhs=wp1[:],
                                    start=(f == 0), stop=(f == NF - 1))
                        for tj in range(4):
                            for half in range(2):
                                cstart = 1024 * n2 + 512 * half
                                ov = wpool.tile([128, TQ], f32, tag="yraw")
                                nc.vector.tensor_tensor(
                                    ov[:], o_ps[2 * tj + half][:],
                                    xm_tm[:, tj, cstart:cstart + 512], Alu.add)
                                nc.sync.dma_start(
                                    out_tm[128 * tj:128 * (tj + 1),
                                           cstart:cstart + 512], ov[:])

    nc.compile()
    return nc


def _make_in_maps(x, cos, sin, weights_b):
    import ml_dtypes
    bf = ml_dtypes.bfloat16
    cosT = cos[0, :, 0, :].T  # [64, T]
    sinT = sin[0, :, 0, :].T
    in_maps = []
    for c in range(NCORES):
        b, r = divmod(c, 4)
        sl = slice(TQ * r, TQ * (r + 1))
        qpos = np.arange(TQ * r, TQ * (r + 1))
        m = {
            "xT": np.ascontiguousarray(x[b, sl, :].T),
            "csc": np.ascontiguousarray(
                np.concatenate([cosT[:, sl], cosT[:, sl]], axis=0)),
            "css": np.ascontiguousarray(
                np.concatenate([sinT[:, sl], -sinT[:, sl]], axis=0)),
            "mask": (np.arange(T)[:, None] <= qpos[None, :]).astype(bf),
        }
        m.update(weights_b)
        in_maps.append(m)
    return in_maps


def kernel(x, cos, sin, wq, wk, wv, wo, w_fc, w_proj):
    global _CACHE
    import ml_dtypes
    from concourse.bass_utils import run_bass_kernel_spmd

    bf = ml_dtypes.bfloat16
    x = np.asarray(x, np.float32)
    cos = np.asarray(cos, np.float32)
    sin = np.asarray(sin, np.float32)
    weights_b = {
        "wq": np.asarray(wq, np.float32).astype(bf),
        "wk": np.asarray(wk, np.float32).astype(bf),
        "wv": np.asarray(wv, np.float32).astype(bf),
        "wo": np.asarray(wo, np.float32).astype(bf),
        "wfc": np.asarray(w_fc, np.float32).astype(bf),
        "wproj": np.asarray(w_proj, np.float32).astype(bf),
    }

    if _CACHE is None:
        _CACHE = _build()
    nc = _CACHE

    in_maps = _make_in_maps(x, cos, sin, weights_b)
    res = run_bass_kernel_spmd(nc, in_maps, list(range(NCORES)))
    out = np.empty((B, T, C), np.float32)
    for c in range(NCORES):
        b, r = divmod(c, 4)
        out[b, TQ * r:TQ * (r + 1), :] = res.results[c]["out"]
    return out


# revision 18
# speedup vs baseline: 1.1224x; 1.0190x over previous
"""Trainium2 Bass kernel for nn_Block_32762010534337 (dense transformer block).

Strategy: sequence-parallel over 8 cores. Core c owns 512 tokens (batch c//4,
token chunk c%4). Each core computes rmsnorm -> K/V projections (+rope, k-norm)
for its own tokens, AllGathers K/V within its batch group of 4 cores (overlapped
with the Q projections), then runs causal attention + wo + MLP (relu^2) for its
512 tokens with fully replicated bf16 weights. Activations stay feature-major
([channel, token]); the host transposes per-core inputs and the final residual
path switches to token-major via 64 PE transposes.
"""
import sys
import os

if "/opt/trn_rl_repo" not in sys.path:
    sys.path.insert(0, "/opt/trn_rl_repo")

import numpy as np

B, T, C = 2, 2048, 2048
NH, NKV, HD = 16, 4, 128
DFF = 4 * C
TQ = 512          # tokens per core
NT = C // 128     # 16 feature tiles
NF = DFF // 128   # 64 ff tiles
EPS = 1.1920929e-07
NCORES = 8

_CACHE = None


def _build():
    import concourse.bass as bass
    import concourse.tile as tile
    from concourse import mybir, bacc
    from concourse.masks import make_identity

    dt = mybir.dt
    f32, bf16, fp8 = dt.float32, dt.bfloat16, dt.float8e4
    Alu = mybir.AluOpType
    Act = mybir.ActivationFunctionType

    nc = bacc.Bacc("TRN2", target_bir_lowering=False, debug=False, num_devices=NCORES)

    for val in (EPS, HD * EPS):
        tns = nc.alloc_sbuf_tensor(f"const-f32-{val}", [128, 1], f32)
        nc.gpsimd.memset(tns.ap(), val)
        nc.const_aps.aps[(f32, val)] = tns.ap()
    nc.all_engine_barrier()

    xT = nc.declare_dram_parameter("xT", [C, TQ], f32, isOutput=False)
    csc = nc.declare_dram_parameter("csc", [128, TQ], f32, isOutput=False)
    css = nc.declare_dram_parameter("css", [128, TQ], f32, isOutput=False)
    mask = nc.declare_dram_parameter("mask", [T, TQ], fp8, isOutput=False)
    wq = nc.declare_dram_parameter("wq", [C, C], bf16, isOutput=False)
    wk = nc.declare_dram_parameter("wk", [C, NKV * HD], bf16, isOutput=False)
    wv = nc.declare_dram_parameter("wv", [C, NKV * HD], bf16, isOutput=False)
    wo = nc.declare_dram_parameter("wo", [C, C], bf16, isOutput=False)
    wfc = nc.declare_dram_parameter("wfc", [C, DFF], bf16, isOutput=False)
    wproj = nc.declare_dram_parameter("wproj", [DFF, C], bf16, isOutput=False)
    out_tm = nc.declare_dram_parameter("out", [TQ, C], f32, isOutput=True)

    ck_in = nc.dram_tensor("ck_in", [512, TQ], bf16)
    ck_out = nc.dram_tensor("ck_out", [2048, TQ], bf16)
    cv_in = nc.dram_tensor("cv_in", [512, TQ], bf16)
    cv_out = nc.dram_tensor("cv_out", [2048, TQ], bf16)

    with tile.TileContext(nc, num_cores=NCORES) as tc:
        with (
            tc.tile_pool(name="const", bufs=1) as constp,
            tc.tile_pool(name="persist", bufs=1) as pp,
            tc.tile_pool(name="work", bufs=3) as wpool,
            tc.tile_pool(name="wstream", bufs=3) as wsp,
        ):
            ident = constp.tile([128, 128], bf16, tag="ident")
            make_identity(nc, ident)
            ident_f = constp.tile([128, 128], f32, tag="identf")
            make_identity(nc, ident_f)
            ones = constp.tile([128, 1], bf16, tag="ones")
            nc.gpsimd.memset(ones, 1.0)
            csc_sb = constp.tile([128, TQ], f32, tag="csc")
            nc.sync.dma_start(csc_sb[:], csc[:])
            css_sb = constp.tile([128, TQ], f32, tag="css")
            nc.sync.dma_start(css_sb[:], css[:])

            # x_mid^T lives across attention + MLP
            xmT = pp.tile([128, NT, TQ], f32, tag="xmT")

            def norm_scale_row(ssq_ps, scale, bias, tag):
                """[1,TQ] psum sum-of-squares -> broadcast [128,TQ] f32 scale."""
                sr = wpool.tile([1, TQ], f32, tag="srow")
                nc.scalar.activation(sr[:], ssq_ps[:], Act.Sqrt, bias=bias, scale=scale)
                sb0 = wpool.tile([128, TQ], f32, tag=tag)
                nc.gpsimd.partition_broadcast(sb0[:], sr[:])
                sb = wpool.tile([128, TQ], f32, tag=tag)
                nc.vector.reciprocal(sb[:], sb0[:])
                return sb

            def load_xT(i, pool):
                xin = pool.tile([128, TQ], f32, tag="xin", bufs=3, name="xin")
                nc.sync.dma_start(xin[:], xT[128 * i:128 * (i + 1), :])
                return xin

            def rope(ps, pool):
                """psum [128,TQ] f32 -> rope'd bf16 sbuf tile."""
                raw = pool.tile([128, TQ], bf16, tag="rraw", bufs=4, name="rraw")
                nc.scalar.copy(raw[:], ps[:])
                sw = pool.tile([128, TQ], bf16, tag="rsw", bufs=2, name="rsw")
                nc.sync.dma_start(sw[0:64, :], raw[64:128, :])
                nc.sync.dma_start(sw[64:128, :], raw[0:64, :])
                rr = pool.tile([128, TQ], bf16, tag="rr", bufs=6, name="rr")
                nc.vector.tensor_tensor(rr[:], raw[:], csc_sb[:], Alu.mult)
                t2 = pool.tile([128, TQ], bf16, tag="rt2", bufs=2, name="rt2")
                nc.vector.tensor_tensor(t2[:], sw[:], css_sb[:], Alu.mult)
                nc.vector.tensor_tensor(rr[:], rr[:], t2[:], Alu.add)
                return rr

            def sumsq(rr, pool):
                sq = pool.tile([128, TQ], bf16, tag="rsq", bufs=6, name="rsq")
                nc.vector.tensor_tensor(sq[:], rr[:], rr[:], Alu.mult)
                return sq

            with tc.tile_pool(name="attn", bufs=1) as ap_:
                mask_sb = ap_.tile([128, NT, TQ], fp8, tag="mask_sb")
                nc.sync.dma_start(mask_sb[:], mask.rearrange("(g p) t -> p g t", p=128))
                qs_sb = ap_.tile([128, NH, TQ], bf16, tag="qs_sb")
                hT = ap_.tile([128, NT, TQ], bf16, tag="hT")
                vloc = ap_.tile([128, 4, TQ], bf16, tag="vloc")

                with tc.tile_pool(name="ps1", bufs=1, space="PSUM") as ps1:
                    # ---- P0: pre-attention rmsnorm (feature-major) ----
                    ssq_ps = ps1.tile([1, TQ], f32, tag="row", bufs=3)
                    for i in range(NT):
                        xin = load_xT(i, ap_)
                        xsq = wpool.tile([128, TQ], bf16, tag="xsq", bufs=8)
                        nc.vector.tensor_tensor(xsq[:], xin[:], xin[:], Alu.mult)
                        nc.tensor.matmul(ssq_ps[:], lhsT=ones[:], rhs=xsq[:],
                                         start=(i == 0), stop=(i == NT - 1))
                    s1b = norm_scale_row(ssq_ps, 1.0 / C, EPS, "sbcast")
                    for i in range(NT):
                        xin = load_xT(i, ap_)
                        nc.vector.tensor_tensor(hT[:, i], xin[:], s1b[:], Alu.mult)

                    # ---- K heads first: project + rope + k-norm -> cc_in ----
                    kps = [ps1.tile([128, TQ], f32, tag="qkv", bufs=4,
                                    name=f"kps_{_k}") for _k in range(4)]
                    for i in range(NT):
                        wk_sb = wsp.tile([128, TQ], bf16, tag="wq_sb")
                        nc.sync.dma_start(wk_sb[:], wk[128 * i:128 * (i + 1), :])
                        for k in range(4):
                            nc.tensor.matmul(kps[k][:],
                                             lhsT=wk_sb[:, 128 * k:128 * (k + 1)],
                                             rhs=hT[:, i],
                                             start=(i == 0), stop=(i == NT - 1))
                    for kh in range(4):
                        rr = rope(kps[kh], ap_)
                        sq = sumsq(rr, ap_)
                        sps = ps1.tile([1, TQ], f32, tag="row", bufs=3)
                        nc.tensor.matmul(sps[:], lhsT=ones[:], rhs=sq[:],
                                         start=True, stop=True)
                        sb = norm_scale_row(sps, 1.0 / HD, EPS, "sbcast")
                        kt = ap_.tile([128, TQ], bf16, tag="ktile", bufs=3, name="kt")
                        nc.vector.tensor_tensor(kt[:], rr[:], sb[:], Alu.mult)
                        nc.sync.dma_start(ck_in[128 * kh:128 * (kh + 1), :], kt[:])

                    nc.gpsimd.collective_compute(
                        "AllGather", Alu.bypass,
                        replica_groups=[[0, 1, 2, 3], [4, 5, 6, 7]],
                        ins=[ck_in[:]], outs=[ck_out[:]])

                    # ---- V heads: project + transpose to token-major -> cv_in ----
                    vps = [ps1.tile([128, TQ], f32, tag="qkv", bufs=4,
                                    name=f"vps_{_k}") for _k in range(4)]
                    for i in range(NT):
                        wv_sb = wsp.tile([128, TQ], bf16, tag="wq_sb")
                        nc.sync.dma_start(wv_sb[:], wv[128 * i:128 * (i + 1), :])
                        for k in range(4):
                            nc.tensor.matmul(vps[k][:],
                                             lhsT=wv_sb[:, 128 * k:128 * (k + 1)],
                                             rhs=hT[:, i],
                                             start=(i == 0), stop=(i == NT - 1))
                    for kh in range(4):
                        vb = ap_.tile([128, TQ], bf16, tag="ktile", bufs=3, name="vb")
                        nc.scalar.copy(vb[:], vps[kh][:])
                        for j in range(4):
                            tps = ps1.tile([128, 128], bf16, tag="tr", bufs=1)
                            nc.tensor.transpose(tps[:], vb[:, 128 * j:128 * (j + 1)],
                                                ident[:])
                            nc.vector.tensor_copy(
                                out=vloc[:, j, 128 * kh:128 * (kh + 1)], in_=tps[:])
                    for j in range(4):
                        nc.sync.dma_start(
                            cv_in[128 * j:128 * (j + 1), :], vloc[:, j, :])
                    nc.gpsimd.collective_compute(
                        "AllGather", Alu.bypass,
                        replica_groups=[[0, 1, 2, 3], [4, 5, 6, 7]],
                        ins=[cv_in[:]], outs=[cv_out[:]])

                    # ---- Q heads: project + rope + deferred q-norm ----
                    # ssq matmuls of group g are emitted after group g+1's
                    # projection matmuls so the PE never waits on the DVE chain
                    pending = None

                    def finish_q(pend):
                        hg, rrs, sqs = pend
                        for k in range(4):
                            h = 4 * hg + k
                            sps = ps1.tile([1, TQ], f32, tag="row", bufs=3)
                            nc.tensor.matmul(sps[:], lhsT=ones[:], rhs=sqs[k][:],
                                             start=True, stop=True)
                            sb = norm_scale_row(sps, 1.0, HD * EPS, "sbcast")
                            nc.vector.tensor_tensor(qs_sb[:, h], rrs[k][:], sb[:],
                                                    Alu.mult)

                    for hg in range(4):
                        qps = [ps1.tile([128, TQ], f32, tag="qkv", bufs=4,
                                        name=f"qps{hg}_{_k}") for _k in range(4)]
                        wts = []
                        for i in range(NT):
                            wq_sb = wsp.tile([128, TQ], bf16, tag="wtile", bufs=24,
                                             name=f"wq{hg}_{i}")
                            nc.sync.dma_start(wq_sb[:], wq[128 * i:128 * (i + 1),
                                                           TQ * hg:TQ * (hg + 1)])
                            wts.append(wq_sb)
                        rrs = []
                        sqs = []
                        for k in range(4):
                            for i in range(NT):
                                nc.tensor.matmul(qps[k][:],
                                                 lhsT=wts[i][:, 128 * k:128 * (k + 1)],
                                                 rhs=hT[:, i],
                                                 start=(i == 0), stop=(i == NT - 1))
                            rrs.append(rope(qps[k], ap_))
                            sqs.append(sumsq(rrs[k], ap_))
                        if pending is not None:
                            finish_q(pending)
                        pending = (hg, rrs, sqs)
                    finish_q(pending)

                # ---- load gathered K/V ----
                k_sb = ap_.tile([128, 16, TQ], bf16, tag="k_sb")   # (kh, g)
                v_sb = ap_.tile([128, 16, TQ], bf16, tag="v_sb")   # (g, j)
                for g in range(4):
                    for kh in range(4):
                        nc.sync.dma_start(
                            k_sb[:, 4 * kh + g],
                            ck_out[512 * g + 128 * kh:512 * g + 128 * (kh + 1), :])
                    for j in range(4):
                        nc.sync.dma_start(
                            v_sb[:, 4 * g + j],
                            cv_out[512 * g + 128 * j:512 * g + 128 * (j + 1), :])

                # ---- attention, 4 sibling q-heads per kv head together ----
                yT = ap_.tile([128, NH, TQ], bf16, tag="hT")
                with tc.tile_pool(name="ps2", bufs=1, space="PSUM") as ps2:
                    for kh in range(NKV):
                      for pr in range(2):
                        hs = [4 * kh + 2 * pr + k for k in range(2)]
                        den2 = ps2.tile([33, TQ], f32, tag="den", bufs=1)
                        y_ps = [ps2.tile([128, TQ], f32, tag="y", bufs=4,
                                         name=f"y{kh}{pr}_{_k}") for _k in range(2)]
                        fifo = []

                        def drain_one():
                            m0, k0, p0 = fifo.pop(0)
                            nc.tensor.matmul(den2[32 * k0:32 * k0 + 1, :],
                                             lhsT=ones[:], rhs=p0[:],
                                             start=(m0 == 0), stop=(m0 == 15))
                            nc.tensor.matmul(
                                y_ps[k0][:],
                                lhsT=v_sb[:, m0, 128 * kh:128 * (kh + 1)],
                                rhs=p0[:],
                                start=(m0 == 0), stop=(m0 == 15))

                        for m in range(16):
                            g, mm = divmod(m, 4)
                            for k in range(2):
                                sc_ps = ps2.tile([128, TQ], f32, tag="sc", bufs=3)
                                nc.tensor.matmul(
                                    sc_ps[:],
                                    lhsT=k_sb[:, 4 * kh + g, 128 * mm:128 * (mm + 1)],
                                    rhs=qs_sb[:, hs[k]], start=True, stop=True)
                                p_bf = ap_.tile([128, TQ], bf16, tag="p_bf",
                                                bufs=7, name="p_bf")
                                nc.scalar.activation(p_bf[:], sc_ps[:], Act.Exp)
                                nc.vector.tensor_tensor(p_bf[:], p_bf[:],
                                                        mask_sb[:, m], Alu.mult)
                                fifo.append((m, k, p_bf))
                                if len(fifo) > 4:
                                    drain_one()
                        while fifo:
                            drain_one()
                        for k in range(2):
                            yraw = wpool.tile([128, TQ], f32, tag="yraw", bufs=3)
                            nc.scalar.copy(yraw[:], y_ps[k][:])
                            dr = wpool.tile([1, TQ], f32, tag="srow")
                            nc.scalar.copy(dr[:], den2[32 * k:32 * k + 1, :])
                            db0 = wpool.tile([128, TQ], f32, tag="sbcast")
                            nc.gpsimd.partition_broadcast(db0[:], dr[:])
                            db = wpool.tile([128, TQ], f32, tag="sbcast")
                            nc.vector.reciprocal(db[:], db0[:])
                            nc.vector.tensor_tensor(yT[:, hs[k]], yraw[:], db[:],
                                                    Alu.mult)

                    # ---- wo projection + residual (feature-major xmT) ----
                    for n4 in range(4):
                        att_ps = [ps2.tile([128, TQ], f32, tag="y", bufs=4,
                                           name=f"att{n4}_{_k}") for _k in range(4)]
                        wts = []
                        for h in range(NH):
                            wo_sb = wsp.tile([128, TQ], bf16, tag="wtile", bufs=24,
                                             name=f"wo{n4}_{h}")
                            nc.sync.dma_start(wo_sb[:], wo[128 * h:128 * (h + 1),
                                                           TQ * n4:TQ * (n4 + 1)])
                            wts.append(wo_sb)
                        for k in range(4):
                            for h in range(NH):
                                nc.tensor.matmul(att_ps[k][:],
                                                 lhsT=wts[h][:, 128 * k:128 * (k + 1)],
                                                 rhs=yT[:, h],
                                                 start=(h == 0), stop=(h == NH - 1))
                            n = 4 * n4 + k
                            xin = load_xT(n, ap_)
                            nc.vector.tensor_tensor(xmT[:, n], att_ps[k][:],
                                                    xin[:], Alu.add)
            # attn pool closed

            # ---- MLP ----
            with tc.tile_pool(name="mlp", bufs=1) as mp:
                h2T = mp.tile([128, NT, TQ], bf16, tag="h2T")
                a_sb = mp.tile([128, NF, TQ], bf16, tag="a_sb")
                xm_tm = mp.tile([128, 4, C], bf16, tag="xm_tm")  # token-major x_mid

                with tc.tile_pool(name="ps3", bufs=1, space="PSUM") as ps3:
                    ssq2 = ps3.tile([1, TQ], f32, tag="row", bufs=2)
                    _dummy = 0
                    for i in range(NT):
                        xsq = wpool.tile([128, TQ], bf16, tag="xsq", bufs=8)
                        nc.vector.tensor_tensor(xsq[:], xmT[:, i], xmT[:, i],
                                                Alu.mult)
                        nc.tensor.matmul(ssq2[:], lhsT=ones[:], rhs=xsq[:],
                                         start=(i == 0), stop=(i == NT - 1))
                    s2b = norm_scale_row(ssq2, 1.0 / C, EPS, "sbcast")
                    for i in range(NT):
                        nc.vector.tensor_tensor(h2T[:, i], xmT[:, i], s2b[:],
                                                Alu.mult)

                    # transpose xmT -> token-major for the final residual
                    for i in range(NT):
                        for j in range(4):
                            tp2 = ps3.tile([128, 128], f32, tag="mm", bufs=6)
                            nc.tensor.transpose(tp2[:],
                                                xmT[:, i, 128 * j:128 * (j + 1)],
                                                ident_f[:])
                            nc.vector.tensor_copy(
                                out=xm_tm[:, j, 128 * i:128 * (i + 1)], in_=tp2[:])

                # fc + relu^2 (feature-major a)
                with tc.tile_pool(name="ps3b", bufs=1, space="PSUM") as ps3b:
                    for jc in range(16):
                        f_ps = [ps3b.tile([128, TQ], f32, tag="mm", bufs=8,
                                          name=f"fps{jc}_{_k}") for _k in range(4)]
                        wts = []
                        for i in range(NT):
                            wfc_sb = wsp.tile([128, TQ], bf16, tag="wtile", bufs=24,
                                              name=f"wfc{jc}_{i}")
                            nc.sync.dma_start(wfc_sb[:], wfc[128 * i:128 * (i + 1),
                                                            TQ * jc:TQ * (jc + 1)])
                            wts.append(wfc_sb)
                        for jf in range(4):
                            for i in range(NT):
                                nc.tensor.matmul(
                                    f_ps[jf][:],
                                    lhsT=wts[i][:, 128 * jf:128 * (jf + 1)],
                                    rhs=h2T[:, i],
                                    start=(i == 0), stop=(i == NT - 1))
                            f = 4 * jc + jf
                            r_bf = wpool.tile([128, TQ], bf16, tag="r_bf")
                            nc.scalar.activation(r_bf[:], f_ps[jf][:], Act.Relu)
                            nc.vector.tensor_tensor(a_sb[:, f], r_bf[:], r_bf[:],
                                                    Alu.mult)

                # proj: lhsT = a tile (1 LDW : 2 MMs), token-major output
                with tc.tile_pool(name="ps4", bufs=1, space="PSUM") as ps4:
                    for n2 in range(2):
                        o_ps = [ps4.tile([128, TQ], f32, tag="o", bufs=8,
                                         name=f"ops{n2}_{_k}") for _k in range(8)]
                        for f in range(NF):
                            wp0 = wsp.tile([128, TQ], bf16, tag="wp_sb")
                            nc.sync.dma_start(
                                wp0[:], wproj[128 * f:128 * (f + 1),
                                              1024 * n2:1024 * n2 + 512])
                            wp1 = wsp.tile([128, TQ], bf16, tag="wp_sb")
                            nc.sync.dma_start(
                                wp1[:], wproj[128 * f:128 * (f + 1),
                                              1024 * n2 + 512:1024 * (n2 + 1)])
                            for tj in range(4):
                                nc.tensor.matmul(
                                    o_ps[2 * tj][:],
                                    lhsT=a_sb[:, f, 128 * tj:128 * (tj + 1)],
                                    rhs=wp0[:],
                                    start=(f == 0), stop=(f == NF - 1))
                                nc.tensor.matmul(
                                    o_ps[2 * tj + 1][:],
                                    lhsT=a_sb[:, f, 128 * tj:128 * (tj + 1)],
                                    rhs=wp1[:],
                                    start=(f == 0), stop=(f == NF - 1))
                        for tj in range(4):
                            for half in range(2):
                                cstart = 1024 * n2 + 512 * half
                                ov = wpool.tile([128, TQ], f32, tag="yraw")
                                nc.vector.tensor_tensor(
                                    ov[:], o_ps[2 * tj + half][:],
                                    xm_tm[:, tj, cstart:cstart + 512], Alu.add)
                                nc.sync.dma_start(
                                    out_tm[128 * tj:128 * (tj + 1),
                                           cstart:cstart + 512], ov[:])

    nc.compile()
    return nc


def _make_in_maps(x, cos, sin, weights_b):
    import ml_dtypes
    cosT = cos[0, :, 0, :].T  # [64, T]
    sinT = sin[0, :, 0, :].T
    in_maps = []
    for c in range(NCORES):
        b, r = divmod(c, 4)
        sl = slice(TQ * r, TQ * (r + 1))
        qpos = np.arange(TQ * r, TQ * (r + 1))
        m = {
            "xT": np.ascontiguousarray(x[b, sl, :].T),
            "csc": np.ascontiguousarray(
                np.concatenate([cosT[:, sl], cosT[:, sl]], axis=0)),
            "css": np.ascontiguousarray(
                np.concatenate([sinT[:, sl], -sinT[:, sl]], axis=0)),
            "mask": (np.arange(T)[:, None] <= qpos[None, :]).astype(ml_dtypes.float8_e4m3),
        }
        m.update(weights_b)
        in_maps.append(m)
    return in_maps


def kernel(x, cos, sin, wq, wk, wv, wo, w_fc, w_proj):
    global _CACHE
    import ml_dtypes
    from concourse.bass_utils import run_bass_kernel_spmd

    bf = ml_dtypes.bfloat16
    x = np.asarray(x, np.float32)
    cos = np.asarray(cos, np.float32)
    sin = np.asarray(sin, np.float32)
    weights_b = {
        "wq": np.asarray(wq, np.float32).astype(bf),
        "wk": np.asarray(wk, np.float32).astype(bf),
        "wv": np.asarray(wv, np.float32).astype(bf),
        "wo": np.asarray(wo, np.float32).astype(bf),
        "wfc": np.asarray(w_fc, np.float32).astype(bf),
        "wproj": np.asarray(w_proj, np.float32).astype(bf),
    }

    if _CACHE is None:
        _CACHE = _build()
    nc = _CACHE

    in_maps = _make_in_maps(x, cos, sin, weights_b)
    res = run_bass_kernel_spmd(nc, in_maps, list(range(NCORES)))
    out = np.empty((B, T, C), np.float32)
    for c in range(NCORES):
        b, r = divmod(c, 4)
        out[b, TQ * r:TQ * (r + 1), :] = res.results[c]["out"]
    return out
